# revision 11
# baseline (speedup 1.0000x reference)
"""ClusterGCN layer on 8 TRN2 NeuronCores.

Math: for each cluster c (only intra-cluster edges matter),
    Y_c = B_c @ (X_c @ W) + b
    B_c[d, s] = dis[d] * At_c[s, d] * dis[s]
    At_c[s, d] = #edges(s->d in c) + [s == d]     (self-loop: dis^2 = 1/deg)
with dis = rsqrt(deg), deg = intra in-degree + 1. Clusters with no intra
edge pass X through unchanged (patched on host).

Device per core — AGGREGATE-FIRST order, so every matmul streams the
wide d axis (N=DN=500) and LDWEIGHTS fully hides under the moving pass:
  phase A: G_c[k, d] = sum_s (X*dis)[s, k] * At_c[s, d]
           stationary = X chunks [128 s, 128 k] fp16 (natural layout),
           moving = At rows fp8 (integer counts <= 16 are exact in e4m3).
  phase B: Y_c^T[f, d] = sum_k W[k, f] * G_c[k, d]
           stationary = W chunks (shared across clusters), moving = G.
Phase B is software-pipelined one cluster behind phase A so the PE never
waits on the G PSUM->SBUF casts (scalar engine; Y casts on vector).
dis[s] is folded into X on the host; dis[d] and the bias are applied in
the host-side gather. All X/At input is prefetched into SBUF in growing
groups on separate DMA queues (sync=X+W, gpsimd=At then YT stores).
"""

import os

import numpy as np

N_CORES = 8
N_CLUSTERS = 100
P = 128

# compute dtype for X/W/G tiles: fp16 matches bf16 PE throughput with 4x
# the mantissa (all values here are O(1), so fp16 range is safe)
_X_DT = os.environ.get("KX_DTYPE", "fp16")
_Y_DT = os.environ.get("KYT_DTYPE", "fp16")

_prog_cache: dict = {}


def _groups(n, first_sizes=(1, 2, 4)):
    """Split range(n) into growing groups: small first loads so compute
    starts early, then bulk. Returns [(start, size), ...]."""
    sizes = []
    c0 = 0
    for g in first_sizes:
        if c0 >= n:
            break
        g = min(g, n - c0)
        sizes.append((c0, g))
        c0 += g
    if c0 < n:
        sizes.append((c0, n - c0))
    return sizes


def _build_program(cpc: int, cap: int, dn: int, in_c: int, f_out: int,
                   fp8_path: bool):
    """Build + compile the per-core Bass program.

    cpc: clusters per core; cap: padded cluster size (multiple of 128);
    dn: real (un-padded) d-column count per cluster.
    fp8_path: adjacency as fp8e4m3 counts (default); the bf16 fallback
    ships pre-scaled B^T blocks (counts > 16 only).
    """
    import concourse.mybir as mybir
    import concourse.tile as tile
    from concourse import bacc

    key = (cpc, cap, dn, in_c, f_out, fp8_path, _X_DT, _Y_DT)
    if key in _prog_cache:
        return _prog_cache[key]

    kc = in_c // P           # k chunks (input channels)
    sch = cap // P           # s chunks per cluster
    fc = f_out // P          # f chunks (output partitions)
    f32 = mybir.dt.float32
    f32r = mybir.dt.float32r
    dt_of = {"bf16": mybir.dt.bfloat16, "fp16": mybir.dt.float16,
             "f32": f32, "f32r": f32r}
    fp8 = mybir.dt.float8e4
    x_dt = dt_of[_X_DT]
    a_dt = fp8 if fp8_path else x_dt

    nc = bacc.Bacc("TRN2", target_bir_lowering=False, debug=False,
                   num_devices=N_CORES)

    # all partition-major so every DMA run is one long contiguous blob
    XN = nc.dram_tensor("XN", [P, cpc, sch, in_c], x_dt,
                        kind="ExternalInput")
    Wt = nc.dram_tensor("Wt", [in_c, f_out], x_dt, kind="ExternalInput")
    AT = nc.dram_tensor("AT", [P, cpc, sch, dn], a_dt, kind="ExternalInput")
    y_dt = dt_of[_Y_DT]
    YT = nc.dram_tensor("YT", [P, cpc, fc, dn], y_dt, kind="ExternalOutput")

    Wtr = Wt.rearrange("(k p) f -> p k f", p=P)

    with tile.TileContext(nc) as tc:
        with (
            tc.tile_pool(name="w", bufs=1) as w_pool,
            tc.tile_pool(name="xn", bufs=1) as xn_pool,
            tc.tile_pool(name="at", bufs=1) as at_pool,
            tc.tile_pool(name="g", bufs=3 * 2) as g_pool,
            tc.tile_pool(name="out", bufs=4) as out_pool,
            tc.tile_pool(name="psg", bufs=3, space="PSUM") as psg_pool,
            tc.tile_pool(name="psy", bufs=4, space="PSUM") as psy_pool,
            tc.tile_pool(name="pswarm", bufs=1, space="PSUM") as pswarm_pool,
        ):
            # per-cluster loads in exact consumption order: the DMA
            # engines drain each queue FIFO, so big tail groups would
            # delay the early clusters (head-of-line blocking).
            # Cluster 0 is split per s-chunk so the first matmul only
            # waits on 1/sch of the data.
            xn = xn_pool.tile([P, cpc, sch, in_c], x_dt)
            for s in range(sch):
                nc.sync.dma_start(xn[:, 0, s], XN[:, 0, s])
            wt = w_pool.tile([P, kc, f_out], x_dt)
            nc.sync.dma_start(wt[:], Wtr[:])
            for c in range(1, cpc):
                nc.sync.dma_start(xn[:, c], XN[:, c])

            # At on the gpsimd queue (later also the YT stores)
            at = at_pool.tile([P, cpc, sch, dn], a_dt)
            for s in range(sch):
                nc.gpsimd.dma_start(at[:, 0, s], AT[:, 0, s])
            for c in range(1, cpc):
                nc.gpsimd.dma_start(at[:, c], AT[:, c])

            # HAM pre-warm: the PE clock-gate needs ~3.4us of sustained
            # activity to release 1.2->2.4 GHz; burn the initial DMA
            # wait on dummy matmuls so the real stream starts warm
            warm = w_pool.tile([P, 64], x_dt)
            nc.vector.memset(warm[:], 0.0)
            wps = pswarm_pool.tile([P, 512], f32)
            for _ in range(45):
                nc.tensor.matmul(wps[:64, :64], lhsT=warm[:], rhs=warm[:],
                                 start=True, stop=True)

            # phase B runs one cluster behind phase A so the PE never
            # waits on the G casts
            g_of = {}

            def phase_a(c):
                # s-outer so the first matmul of cluster 0 only needs
                # the first s-chunk of X/At; psg[k] accumulation groups
                # interleave across the two PSUM banks
                pss = [psg_pool.tile([P, 512], f32, name="psg")
                       for k in range(kc)]
                for s in range(sch):
                    for k in range(kc):
                        nc.tensor.matmul(
                            pss[k][:, :dn],
                            lhsT=xn[:, c, s, k * P:(k + 1) * P],
                            rhs=at[:, c, s, :],
                            start=(s == 0),
                            stop=(s == sch - 1),
                        )
                g_tiles = []
                for k in range(kc):
                    gt = g_pool.tile([P, dn], x_dt)
                    nc.scalar.copy(gt[:], pss[k][:, :dn])
                    g_tiles.append(gt)
                g_of[c] = g_tiles

            def phase_b(c):
                # last cluster: split casts across scalar+vector and
                # split the store so the drain tail is shorter
                last = c == cpc - 1
                g_tiles = g_of.pop(c)
                ot = out_pool.tile([P, fc, dn], y_dt)
                for f in range(fc):
                    ps = psy_pool.tile([P, 512], f32)
                    for k in range(kc):
                        nc.tensor.matmul(
                            ps[:, :dn],
                            lhsT=wt[:, k, f * P:(f + 1) * P],
                            rhs=g_tiles[k][:],
                            start=(k == 0),
                            stop=(k == kc - 1),
                        )
                    if last and f % 2 == 0:
                        nc.scalar.copy(ot[:, f], ps[:, :dn])
                    else:
                        nc.vector.tensor_copy(ot[:, f], ps[:, :dn])
                    if last:
                        nc.gpsimd.dma_start(YT[:, c, f], ot[:, f])
                if not last:
                    nc.gpsimd.dma_start(YT[:, c], ot[:])

            for c in range(cpc):
                phase_a(c)
                if c > 0:
                    phase_b(c - 1)
            phase_b(cpc - 1)

    nc.compile()
    _prog_cache[key] = nc
    return nc


def _host_prep(X, W, b, assign, full_ei):
    """Shard + preprocess. Returns (in_maps, fp8_path, gather info)."""
    n, in_c = X.shape
    f_out = W.shape[1]
    src = full_ei[0].astype(np.int64)
    dst = full_ei[1].astype(np.int64)
    a_s = assign[src]
    intra = a_s == assign[dst]
    es, ed = src[intra], dst[intra]

    deg = np.ones(n, np.float32)
    np.add.at(deg, ed, np.float32(1))
    dis = (1.0 / np.sqrt(deg)).astype(np.float32)

    has_edge = np.zeros(N_CLUSTERS, bool)
    has_edge[np.unique(a_s[intra])] = True

    sizes = np.bincount(assign, minlength=N_CLUSTERS)
    cpc = -(-N_CLUSTERS // N_CORES)            # clusters per core
    cap = max(512, int(-(-sizes.max() // P)) * P)  # padded cluster size
    dn = int(sizes.max())                      # real d columns per cluster

    starts = np.zeros(N_CLUSTERS + 1, np.int64)
    starts[1:] = np.cumsum(sizes)
    order = np.argsort(assign, kind="stable")
    pos = np.empty(n, np.int64)
    pos[order] = np.arange(n) - starts[assign[order]]

    ctot = cpc * N_CORES
    # At blocks: At[c][s, d] = #edges(s->d) + [s==d]
    At = np.zeros((ctot, cap, cap), np.uint16)
    np.add.at(At, (assign[es], pos[es], pos[ed]), 1)
    At[assign, pos, pos] += 1
    fp8_path = int(At.max()) <= 16    # integers <= 16 are exact in e4m3

    import ml_dtypes
    x_np = {"bf16": ml_dtypes.bfloat16, "fp16": np.float16,
            "f32": np.float32, "f32r": np.float32}[_X_DT]

    Xp = np.zeros((ctot, cap, in_c), np.float32)
    if fp8_path:
        # fold the dis[s] row scale into X so the device never scales
        Xp[assign, pos] = X * dis[:, None]
    else:
        Xp[assign, pos] = X
    # [c, s, k] -> [p, c, so, k]: partition-major, contiguous runs
    sch = cap // P
    XN_all = np.ascontiguousarray(
        Xp.reshape(ctot, sch, P, in_c).transpose(2, 0, 1, 3)).astype(x_np)

    DISp = np.zeros((ctot, cap), np.float32)
    DISp[assign, pos] = dis

    if fp8_path:
        import concourse.mybir as mybir
        At_send = At.astype(mybir.dt.np(mybir.dt.float8e4))
    else:
        # rare fallback: pre-scaled B^T blocks in the compute dtype
        At_send = (At.astype(np.float32)
                   * DISp[:, :, None] * DISp[:, None, :]).astype(x_np)
    # [c, s, d] -> [p, c, so, d<dn]: partition-major
    At_send = np.ascontiguousarray(
        At_send.reshape(-1, sch, P, cap).transpose(2, 0, 1, 3)[:, :, :, :dn])

    in_maps = []
    for i in range(N_CORES):
        in_maps.append({
            "XN": np.ascontiguousarray(XN_all[:, i * cpc:(i + 1) * cpc]),
            "Wt": W.astype(np.float32).astype(x_np),
            "AT": np.ascontiguousarray(At_send[:, i * cpc:(i + 1) * cpc]),
        })
    return in_maps, fp8_path, (cpc, cap, dn, has_edge, pos, dis)


def _run(inputs, trace=False, tmpdir=None):
    from concourse.bass_utils import run_bass_kernel_spmd

    X = np.asarray(inputs["X"], np.float32)
    W = np.asarray(inputs["W"], np.float32)
    b = np.asarray(inputs["b"], np.float32)
    assign = np.asarray(inputs["assign"])
    full_ei = np.asarray(inputs["full_ei"])

    n, in_c = X.shape
    f_out = W.shape[1]
    in_maps, fp8_path, (cpc, cap, dn, has_edge, pos, dis) = _host_prep(
        X, W, b, assign, full_ei)
    nc = _build_program(cpc, cap, dn, in_c, f_out, fp8_path)

    res = run_bass_kernel_spmd(
        nc, in_maps, core_ids=list(range(N_CORES)),
        trace=trace, tmpdir=tmpdir,
    )
    # YT: [core][P, cpc, fc, dn]; Y[n, f*128+p] = YT[core, p, lc, f, pos]
    YTdev = np.stack([res.results[i]["YT"] for i in range(N_CORES)])
    if YTdev.dtype != np.float32:
        YTdev = YTdev.astype(np.float32)

    c = assign.astype(np.int64)
    core = c // cpc
    lc = c % cpc
    Y = YTdev[core, :, lc, :, pos]          # [n, P, fc]
    Y = Y.transpose(0, 2, 1).reshape(n, f_out)
    if fp8_path:
        Y = Y * dis[:, None]
    Y += b[None, :].astype(np.float32)
    miss = ~has_edge[c]
    if miss.any():
        Y[miss] = X[miss]
    return Y, res


def kernel(**inputs) -> np.ndarray:
    Y, _ = _run(inputs)
    return Y


# revision 13
# speedup vs baseline: 1.0055x; 1.0055x over previous
"""ClusterGCN layer on 8 TRN2 NeuronCores.

Math: for each cluster c (only intra-cluster edges matter),
    Y_c = B_c @ (X_c @ W) + b
    B_c[d, s] = dis[d] * At_c[s, d] * dis[s]
    At_c[s, d] = #edges(s->d in c) + [s == d]     (self-loop: dis^2 = 1/deg)
with dis = rsqrt(deg), deg = intra in-degree + 1. Clusters with no intra
edge pass X through unchanged (patched on host).

Device per core — AGGREGATE-FIRST order, so every matmul streams the
wide d axis (N=DN=500) and LDWEIGHTS fully hides under the moving pass:
  phase A: G_c[k, d] = sum_s (X*dis)[s, k] * At_c[s, d]
           stationary = X chunks [128 s, 128 k] fp16 (natural layout),
           moving = At rows fp8 (integer counts <= 16 are exact in e4m3).
  phase B: Y_c^T[f, d] = sum_k W[k, f] * G_c[k, d]
           stationary = W chunks (shared across clusters), moving = G.
Phase B is software-pipelined one cluster behind phase A so the PE never
waits on the G PSUM->SBUF casts (scalar engine; Y casts on vector).
dis[s] is folded into X on the host; dis[d] and the bias are applied in
the host-side gather. All X/At input is prefetched into SBUF in growing
groups on separate DMA queues (sync=X+W, gpsimd=At then YT stores).
"""

import os

import numpy as np

N_CORES = 8
N_CLUSTERS = 100
P = 128

# compute dtype for X/W/G tiles: fp16 matches bf16 PE throughput with 4x
# the mantissa (all values here are O(1), so fp16 range is safe)
_X_DT = os.environ.get("KX_DTYPE", "fp16")
_Y_DT = os.environ.get("KYT_DTYPE", "fp16")

_prog_cache: dict = {}


def _groups(n, first_sizes=(1, 2, 4)):
    """Split range(n) into growing groups: small first loads so compute
    starts early, then bulk. Returns [(start, size), ...]."""
    sizes = []
    c0 = 0
    for g in first_sizes:
        if c0 >= n:
            break
        g = min(g, n - c0)
        sizes.append((c0, g))
        c0 += g
    if c0 < n:
        sizes.append((c0, n - c0))
    return sizes


def _build_program(cpc: int, cap: int, dn: int, in_c: int, f_out: int,
                   fp8_path: bool):
    """Build + compile the per-core Bass program.

    cpc: clusters per core; cap: padded cluster size (multiple of 128);
    dn: real (un-padded) d-column count per cluster.
    fp8_path: adjacency as fp8e4m3 counts (default); the bf16 fallback
    ships pre-scaled B^T blocks (counts > 16 only).
    """
    import concourse.mybir as mybir
    import concourse.tile as tile
    from concourse import bacc

    key = (cpc, cap, dn, in_c, f_out, fp8_path, _X_DT, _Y_DT)
    if key in _prog_cache:
        return _prog_cache[key]

    kc = in_c // P           # k chunks (input channels)
    sch = cap // P           # s chunks per cluster
    fc = f_out // P          # f chunks (output partitions)
    f32 = mybir.dt.float32
    f32r = mybir.dt.float32r
    dt_of = {"bf16": mybir.dt.bfloat16, "fp16": mybir.dt.float16,
             "f32": f32, "f32r": f32r}
    fp8 = mybir.dt.float8e4
    x_dt = dt_of[_X_DT]
    a_dt = fp8 if fp8_path else x_dt

    nc = bacc.Bacc("TRN2", target_bir_lowering=False, debug=False,
                   num_devices=N_CORES)

    # all partition-major so every DMA run is one long contiguous blob
    XN = nc.dram_tensor("XN", [P, cpc, sch, in_c], x_dt,
                        kind="ExternalInput")
    Wt = nc.dram_tensor("Wt", [in_c, f_out], x_dt, kind="ExternalInput")
    AT = nc.dram_tensor("AT", [P, cpc, sch, dn], a_dt, kind="ExternalInput")
    y_dt = dt_of[_Y_DT]
    YT = nc.dram_tensor("YT", [P, cpc, fc, dn], y_dt, kind="ExternalOutput")

    Wtr = Wt.rearrange("(k p) f -> p k f", p=P)

    with tile.TileContext(nc) as tc:
        with (
            tc.tile_pool(name="w", bufs=1) as w_pool,
            tc.tile_pool(name="xn", bufs=1) as xn_pool,
            tc.tile_pool(name="at", bufs=1) as at_pool,
            tc.tile_pool(name="g", bufs=3 * 2) as g_pool,
            tc.tile_pool(name="out", bufs=4) as out_pool,
            tc.tile_pool(name="psg", bufs=3, space="PSUM") as psg_pool,
            tc.tile_pool(name="psy", bufs=4, space="PSUM") as psy_pool,
            tc.tile_pool(name="pswarm", bufs=1, space="PSUM") as pswarm_pool,
        ):
            # per-cluster loads in exact consumption order: the DMA
            # engines drain each queue FIFO, so big tail groups would
            # delay the early clusters (head-of-line blocking).
            # Cluster 0 is split per s-chunk so the first matmul only
            # waits on 1/sch of the data.
            xn = xn_pool.tile([P, cpc, sch, in_c], x_dt)
            wt = w_pool.tile([P, kc, f_out], x_dt)
            for c in range(cpc):
                nc.sync.dma_start(xn[:, c], XN[:, c])
                if c == 2:
                    # W rides after cluster 2: it only gates phase B of
                    # cluster 0 (~5 A-phases later), and keeping it out
                    # of the first transfers relieves the early input
                    # pinch where the PE catches up with the wire
                    nc.sync.dma_start(wt[:], Wtr[:])

            # At on the gpsimd queue (later also the YT stores)
            at = at_pool.tile([P, cpc, sch, dn], a_dt)
            for c in range(cpc):
                nc.gpsimd.dma_start(at[:, c], AT[:, c])

            # HAM pre-warm: the PE clock-gate releases 1.2->2.4 GHz only
            # after ~3.4us of SUSTAINED activity, and any idle gap resets
            # the window. Size the dummy-matmul burn (~52ns each) to
            # bridge engine-start to first-data with zero idle, so the
            # gate flips before the first real matmul issues.
            warm = w_pool.tile([P, 64], x_dt)
            nc.vector.memset(warm[:], 0.0)
            wps = pswarm_pool.tile([P, 512], f32)
            for _ in range(70):
                nc.tensor.matmul(wps[:64, :64], lhsT=warm[:], rhs=warm[:],
                                 start=True, stop=True)

            # phase B runs one cluster behind phase A so the PE never
            # waits on the G casts
            g_of = {}

            def phase_a(c):
                g_tiles = []
                for k in range(kc):
                    ps = psg_pool.tile([P, 512], f32)
                    for s in range(sch):
                        nc.tensor.matmul(
                            ps[:, :dn],
                            lhsT=xn[:, c, s, k * P:(k + 1) * P],
                            rhs=at[:, c, s, :],
                            start=(s == 0),
                            stop=(s == sch - 1),
                        )
                    gt = g_pool.tile([P, dn], x_dt)
                    nc.scalar.copy(gt[:], ps[:, :dn])
                    g_tiles.append(gt)
                g_of[c] = g_tiles

            def phase_b(c):
                # last cluster: split casts across scalar+vector and
                # split the store so the drain tail is shorter
                last = c == cpc - 1
                g_tiles = g_of.pop(c)
                ot = out_pool.tile([P, fc, dn], y_dt)
                for f in range(fc):
                    ps = psy_pool.tile([P, 512], f32)
                    for k in range(kc):
                        nc.tensor.matmul(
                            ps[:, :dn],
                            lhsT=wt[:, k, f * P:(f + 1) * P],
                            rhs=g_tiles[k][:],
                            start=(k == 0),
                            stop=(k == kc - 1),
                        )
                    if last and f % 2 == 0:
                        nc.scalar.copy(ot[:, f], ps[:, :dn])
                    else:
                        nc.vector.tensor_copy(ot[:, f], ps[:, :dn])
                    if last:
                        nc.gpsimd.dma_start(YT[:, c, f], ot[:, f])
                if not last:
                    nc.gpsimd.dma_start(YT[:, c], ot[:])

            for c in range(cpc):
                phase_a(c)
                if c > 0:
                    phase_b(c - 1)
            phase_b(cpc - 1)

    nc.compile()
    _prog_cache[key] = nc
    return nc


def _host_prep(X, W, b, assign, full_ei):
    """Shard + preprocess. Returns (in_maps, fp8_path, gather info)."""
    n, in_c = X.shape
    f_out = W.shape[1]
    src = full_ei[0].astype(np.int64)
    dst = full_ei[1].astype(np.int64)
    a_s = assign[src]
    intra = a_s == assign[dst]
    es, ed = src[intra], dst[intra]

    deg = np.ones(n, np.float32)
    np.add.at(deg, ed, np.float32(1))
    dis = (1.0 / np.sqrt(deg)).astype(np.float32)

    has_edge = np.zeros(N_CLUSTERS, bool)
    has_edge[np.unique(a_s[intra])] = True

    sizes = np.bincount(assign, minlength=N_CLUSTERS)
    cpc = -(-N_CLUSTERS // N_CORES)            # clusters per core
    cap = max(512, int(-(-sizes.max() // P)) * P)  # padded cluster size
    dn = int(sizes.max())                      # real d columns per cluster

    starts = np.zeros(N_CLUSTERS + 1, np.int64)
    starts[1:] = np.cumsum(sizes)
    order = np.argsort(assign, kind="stable")
    pos = np.empty(n, np.int64)
    pos[order] = np.arange(n) - starts[assign[order]]

    ctot = cpc * N_CORES
    # At blocks: At[c][s, d] = #edges(s->d) + [s==d]
    At = np.zeros((ctot, cap, cap), np.uint16)
    np.add.at(At, (assign[es], pos[es], pos[ed]), 1)
    At[assign, pos, pos] += 1
    fp8_path = int(At.max()) <= 16    # integers <= 16 are exact in e4m3

    import ml_dtypes
    x_np = {"bf16": ml_dtypes.bfloat16, "fp16": np.float16,
            "f32": np.float32, "f32r": np.float32}[_X_DT]

    Xp = np.zeros((ctot, cap, in_c), np.float32)
    if fp8_path:
        # fold the dis[s] row scale into X so the device never scales
        Xp[assign, pos] = X * dis[:, None]
    else:
        Xp[assign, pos] = X
    # [c, s, k] -> [p, c, so, k]: partition-major, contiguous runs
    sch = cap // P
    XN_all = np.ascontiguousarray(
        Xp.reshape(ctot, sch, P, in_c).transpose(2, 0, 1, 3)).astype(x_np)

    DISp = np.zeros((ctot, cap), np.float32)
    DISp[assign, pos] = dis

    if fp8_path:
        import concourse.mybir as mybir
        At_send = At.astype(mybir.dt.np(mybir.dt.float8e4))
    else:
        # rare fallback: pre-scaled B^T blocks in the compute dtype
        At_send = (At.astype(np.float32)
                   * DISp[:, :, None] * DISp[:, None, :]).astype(x_np)
    # [c, s, d] -> [p, c, so, d<dn]: partition-major
    At_send = np.ascontiguousarray(
        At_send.reshape(-1, sch, P, cap).transpose(2, 0, 1, 3)[:, :, :, :dn])

    in_maps = []
    for i in range(N_CORES):
        in_maps.append({
            "XN": np.ascontiguousarray(XN_all[:, i * cpc:(i + 1) * cpc]),
            "Wt": W.astype(np.float32).astype(x_np),
            "AT": np.ascontiguousarray(At_send[:, i * cpc:(i + 1) * cpc]),
        })
    return in_maps, fp8_path, (cpc, cap, dn, has_edge, pos, dis)


def _run(inputs, trace=False, tmpdir=None):
    from concourse.bass_utils import run_bass_kernel_spmd

    X = np.asarray(inputs["X"], np.float32)
    W = np.asarray(inputs["W"], np.float32)
    b = np.asarray(inputs["b"], np.float32)
    assign = np.asarray(inputs["assign"])
    full_ei = np.asarray(inputs["full_ei"])

    n, in_c = X.shape
    f_out = W.shape[1]
    in_maps, fp8_path, (cpc, cap, dn, has_edge, pos, dis) = _host_prep(
        X, W, b, assign, full_ei)
    nc = _build_program(cpc, cap, dn, in_c, f_out, fp8_path)

    res = run_bass_kernel_spmd(
        nc, in_maps, core_ids=list(range(N_CORES)),
        trace=trace, tmpdir=tmpdir,
    )
    # YT: [core][P, cpc, fc, dn]; Y[n, f*128+p] = YT[core, p, lc, f, pos]
    YTdev = np.stack([res.results[i]["YT"] for i in range(N_CORES)])
    if YTdev.dtype != np.float32:
        YTdev = YTdev.astype(np.float32)

    c = assign.astype(np.int64)
    core = c // cpc
    lc = c % cpc
    Y = YTdev[core, :, lc, :, pos]          # [n, P, fc]
    Y = Y.transpose(0, 2, 1).reshape(n, f_out)
    if fp8_path:
        Y = Y * dis[:, None]
    Y += b[None, :].astype(np.float32)
    miss = ~has_edge[c]
    if miss.any():
        Y[miss] = X[miss]
    return Y, res


def kernel(**inputs) -> np.ndarray:
    Y, _ = _run(inputs)
    return Y
